# revision 15
# baseline (speedup 1.0000x reference)
"""Trainium2 Bass kernel for MQA causal attention with null token.

Problem (reference.py):
  b=4, n=2048, dim=1024, HEADS=16, DIM_HEAD=64
  q  = (x @ Wq).reshape(b,n,16,64).transpose -> [b,h,n,64] * 64**-0.5
  kv = x @ Wkv                                -> [b,n,64]; prepend null -> [b,2049,64]
  sim = q @ kv^T  (causal: query i sees kv cols 0..i+1)
  out = softmax(sim) @ kv -> concat heads -> @ Wout

Sharding: 8 cores = batch(4) x head-half(2). Each core handles one batch
element and 8 heads, computing a partial out-projection; host adds the two
half-head partials per batch.

Device algorithm (per core), all matmuls fp16 inputs w/ fp32 PSUM accumulate:
  XT  [1024,2048]  = x[b]^T                       (host-prepped fp16)
  QT2 [128,2048]x4 = Wq_pair^T @ XT               (head pairs stacked on partitions)
  KVT2[128,2049]   = [Wkv|Wkv]^T @ XT  + null col (kv^T duplicated on both halves)
  KV_aug chunks [128,65] = PE-transpose of KVT2 cols + ones column
  Scores TRANSPOSED (k on partitions): ST[c] = KVT2_chunk^T-contract @ QT2
    -> exp on ACT (PSUM->SBUF fp16) -> causal mask multiply (diag chunks only)
    -> PV: OT_raw[65,512] += KV_aug_chunk^T @ expST   (row 64 = softmax denom)
  normalize: recip(denom) -> partition-broadcast -> multiply -> AT (fp16)
  out = AT^T-contract @ Wout -> fp32 -> HBM
"""

import sys

for _p in ("/opt/trn_rl_repo",):
    if _p not in sys.path:
        sys.path.insert(0, _p)

import numpy as np

HEADS = 16
DH = 64
B = 4
N = 2048
DIM = 1024
NQB = 4          # q blocks of 512 per head
QB = 512
NKC = 16         # full k chunks of 128 (plus 1-row chunk 16)
KTOT = N + 1     # 2049 kv positions (null at 0)

_PROGRAM_CACHE = {}


def _build_program(debug=False, recip_mode="approx_sbuf"):
    import concourse.bacc as bacc
    import concourse.tile as tile
    import concourse.mybir as mybir
    from concourse import bass

    f16 = mybir.dt.float16
    f32 = mybir.dt.float32
    EXP = mybir.ActivationFunctionType.Exp

    nc = bacc.Bacc("TRN2", debug=False, num_devices=8)

    xt_d = nc.dram_tensor("xt", [DIM, N], f16, kind="ExternalInput").ap()
    wq_d = nc.dram_tensor("wq", [DIM, 512], f16, kind="ExternalInput").ap()
    wkv2_d = nc.dram_tensor("wkv2", [DIM, 128], f16, kind="ExternalInput").ap()
    nullkv2_d = nc.dram_tensor("nullkv2", [128, 1], f16, kind="ExternalInput").ap()
    wout_d = nc.dram_tensor("wout", [512, DIM], f16, kind="ExternalInput").ap()
    masks_d = nc.dram_tensor("masks", [128, 4 * QB], f16, kind="ExternalInput").ap()
    ident_d = nc.dram_tensor("ident", [128, 128], f16, kind="ExternalInput").ap()
    out_d = nc.dram_tensor("out", [N, DIM], f32, kind="ExternalOutput").ap()
    dbg = {}
    if debug:
        for name, shape, dt_ in [
            ("dbg_qt2", [128, 4, N], f16),
            ("dbg_kvt2", [128, KTOT], f16),
            ("dbg_kvaug", [128, 17 * 65], f16),
            ("dbg_expst", [128, 1024], f16),
            ("dbg_ot", [65, 512], f32),
            ("dbg_recip", [1, 512], f16),
            ("dbg_bcast", [64, 512], f32),
            ("dbg_at", [128, 4, N], f16),
        ]:
            dbg[name] = nc.dram_tensor(name, shape, dt_, kind="ExternalOutput").ap()

    with tile.TileContext(nc) as tc:
        from contextlib import ExitStack

        with ExitStack() as ctx:
            consts = ctx.enter_context(tc.tile_pool(name="consts", bufs=1))
            work = ctx.enter_context(tc.tile_pool(name="work", bufs=4))
            small = ctx.enter_context(tc.tile_pool(name="small", bufs=2))

            # ---- constant / persistent SBUF tiles ----
            xt_sb = consts.tile([128, 8, N], f16, tag="xt")
            wq_sb = consts.tile([128, 8, 512], f16, tag="wq")
            wkv2_sb = consts.tile([128, 8, 128], f16, tag="wkv2")
            wout_sb = consts.tile([128, 4, DIM], f16, tag="wout")
            masks_sb = consts.tile([128, 4 * QB], f16, tag="masks")
            ident_sb = consts.tile([128, 128], f16, tag="ident")
            kvt2_sb = consts.tile([128, KTOT], f16, tag="kvt2")
            ones_sb = consts.tile([128, 64], f16, tag="ones")
            nc.vector.memset(ones_sb, 1.0)
            kvaug_sb = consts.tile([128, 17 * 65], f16, tag="kvaug")
            qt2_sb = consts.tile([128, 4, N], f16, tag="qt2")
            at_sb = consts.tile([128, 4, N], f16, tag="at")

            nc.sync.dma_start(out=xt_sb, in_=xt_d.rearrange("(d p) t -> p d t", p=128))
            nc.sync.dma_start(out=wq_sb, in_=wq_d.rearrange("(d p) m -> p d m", p=128))
            nc.sync.dma_start(
                out=wkv2_sb, in_=wkv2_d.rearrange("(d p) m -> p d m", p=128)
            )
            nc.sync.dma_start(
                out=wout_sb, in_=wout_d.rearrange("(f p) o -> p f o", p=128)
            )
            nc.sync.dma_start(out=masks_sb, in_=masks_d)
            nc.sync.dma_start(out=ident_sb, in_=ident_d)
            nc.sync.dma_start(out=kvt2_sb[:, 0:1], in_=nullkv2_d)

            # ---- Phase A: projections (QT2 head pairs + KVT2) ----
            with tc.tile_pool(name="proj_ps", bufs=2, space="PSUM") as proj_ps, \
                 tc.tile_pool(name="tr_ps", bufs=2, space="PSUM") as tr_ps:
                for nn in range(4):
                    ts = slice(nn * 512, (nn + 1) * 512)
                    for pair in range(4):
                        qp = proj_ps.tile([128, 512], f32, tag="proj")
                        for d in range(8):
                            nc.tensor.matmul(
                                qp,
                                lhsT=wq_sb[:, d, pair * 128:(pair + 1) * 128],
                                rhs=xt_sb[:, d, ts],
                                start=(d == 0),
                                stop=(d == 7),
                            )
                        nc.scalar.copy(qt2_sb[:, pair, ts], qp)
                    kp = proj_ps.tile([128, 512], f32, tag="proj")
                    for d in range(8):
                        nc.tensor.matmul(
                            kp,
                            lhsT=wkv2_sb[:, d, :],
                            rhs=xt_sb[:, d, ts],
                            start=(d == 0),
                            stop=(d == 7),
                        )
                    nc.vector.tensor_copy(kvt2_sb[:, 1 + nn * 512:513 + nn * 512], kp)

                # ---- Phase B: KV_aug chunks (transpose of KVT2) ----
                for c in range(17):
                    w = 128 if c < 16 else 1
                    tp = tr_ps.tile([128, 64], f16, tag="tr")
                    nc.tensor.transpose(
                        tp[0:w, :],
                        kvt2_sb[0:64, c * 128:c * 128 + w],
                        ident_sb[0:64, 0:64],
                    )
                    nc.vector.tensor_copy(kvaug_sb[0:w, c * 65:c * 65 + 64], tp[0:w, :])
                    nc.vector.memset(kvaug_sb[0:w, c * 65 + 64:c * 65 + 65], 1.0)

            # ---- Phase C: attention ----
            with tc.tile_pool(name="st_ps", bufs=2, space="PSUM") as st_ps, \
                 tc.tile_pool(name="ot_ps", bufs=2, space="PSUM") as ot_ps, \
                 tc.tile_pool(name="edge_ps", bufs=2, space="PSUM") as edge_ps:
                for pair in range(4):
                    for qb in range(NQB):
                        qs = slice(qb * QB, (qb + 1) * QB)
                        nchunks = 4 * qb + 4
                        for parity in range(2):
                            p0 = 64 * parity
                            ot = ot_ps.tile([65, 512], f32, tag="ot")
                            for g in range(nchunks // 2):
                                st = st_ps.tile([128, 1024], f32, tag="st")
                                for i in range(2):
                                    c = 2 * g + i
                                    nc.tensor.matmul(
                                        st[:, i * 512:(i + 1) * 512],
                                        lhsT=kvt2_sb[p0:p0 + 64, c * 128:(c + 1) * 128],
                                        rhs=qt2_sb[p0:p0 + 64, pair, qs],
                                        start=True,
                                        stop=True,
                                    )
                                expst = work.tile([128, 1024], f16, tag="expst")
                                nc.scalar.activation(expst, st, EXP)
                                if debug and pair == 0 and qb == 0 and parity == 0 \
                                        and g == 0:
                                    nc.sync.dma_start(
                                        out=dbg["dbg_expst"], in_=expst
                                    )
                                for i in range(2):
                                    c = 2 * g + i
                                    t = c - 4 * qb
                                    if t >= 0:
                                        nc.vector.tensor_mul(
                                            expst[:, i * 512:(i + 1) * 512],
                                            expst[:, i * 512:(i + 1) * 512],
                                            masks_sb[:, t * QB:(t + 1) * QB],
                                        )
                                for i in range(2):
                                    c = 2 * g + i
                                    nc.tensor.matmul(
                                        ot,
                                        lhsT=kvaug_sb[:, c * 65:c * 65 + 65],
                                        rhs=expst[:, i * 512:(i + 1) * 512],
                                        start=(c == 0),
                                        stop=False,
                                    )
                            # edge element: q = qb*512+511 attends k = qb*512+512
                            cE = 4 * qb + 4
                            kE = 128 * cE
                            es = edge_ps.tile([1, 1], f32, tag="edge")
                            nc.tensor.matmul(
                                es,
                                lhsT=kvt2_sb[p0:p0 + 64, kE:kE + 1],
                                rhs=qt2_sb[p0:p0 + 64, pair, qb * QB + 511:qb * QB + 512],
                                start=True,
                                stop=True,
                            )
                            ee = small.tile([1, 1], f16, tag="edge_sb")
                            nc.scalar.activation(ee, es, EXP)
                            nc.tensor.matmul(
                                ot[:, 511:512],
                                lhsT=kvaug_sb[0:1, cE * 65:cE * 65 + 65],
                                rhs=ee,
                                start=False,
                                stop=True,
                            )
                            # normalize: AT[:, head cols] = OT[0:64] / denom(row 64)
                            # recip on DVE (std op), broadcast across partitions
                            # via PE outer product ones[64] x recip[512]
                            recip = small.tile([65, 512], f16, tag="recip")
                            with nc.allow_low_precision(reason="softmax recip f16"):
                                nc.vector.reciprocal(recip[64:65, :], ot[64:65, :])
                            rep = edge_ps.tile([64, 512], f32, tag="edge")
                            nc.tensor.matmul(
                                rep,
                                lhsT=ones_sb[64:65, :],
                                rhs=recip[64:65, :],
                                start=True,
                                stop=True,
                            )
                            bcast = small.tile([64, 512], f32, tag="bcast")
                            nc.vector.tensor_copy(bcast, rep)
                            if debug and pair == 0 and qb == 0 and parity == 0:
                                otd = small.tile([65, 512], f32, tag="dbg_ot")
                                nc.vector.tensor_copy(otd, ot)
                                nc.sync.dma_start(out=dbg["dbg_ot"], in_=otd)
                                nc.sync.dma_start(
                                    out=dbg["dbg_recip"], in_=recip[64:65, :]
                                )
                                nc.sync.dma_start(out=dbg["dbg_bcast"], in_=bcast)
                            if parity == 0:
                                nc.vector.tensor_mul(
                                    at_sb[0:64, pair, qs], ot[0:64, :], bcast
                                )
                            else:
                                stg = small.tile([64, 512], f16, tag="stg")
                                nc.vector.tensor_mul(stg, ot[0:64, :], bcast)
                                nc.sync.dma_start(
                                    out=at_sb[64:128, pair, qs], in_=stg
                                )

            # ---- Phase D: out projection ----
            with tc.tile_pool(name="op_ps", bufs=2, space="PSUM") as op_ps:
                for m in range(16):
                    ms = slice(m * 128, (m + 1) * 128)
                    for nn in range(2):
                        os_ = slice(nn * 512, (nn + 1) * 512)
                        op = op_ps.tile([128, 512], f32, tag="op")
                        for fc in range(4):
                            nc.tensor.matmul(
                                op,
                                lhsT=at_sb[:, fc, ms],
                                rhs=wout_sb[:, fc, os_],
                                start=(fc == 0),
                                stop=(fc == 3),
                            )
                        ost = work.tile([128, 512], f32, tag="ost")
                        nc.scalar.copy(ost, op)
                        nc.sync.dma_start(out=out_d[ms, os_], in_=ost)

            if debug:
                nc.sync.dma_start(out=dbg["dbg_qt2"], in_=qt2_sb)
                nc.sync.dma_start(out=dbg["dbg_kvt2"], in_=kvt2_sb)
                nc.sync.dma_start(out=dbg["dbg_kvaug"], in_=kvaug_sb)
                nc.sync.dma_start(out=dbg["dbg_at"], in_=at_sb)

    nc.finalize()
    return nc


def _host_prep(x, Wq, Wkv, null_kv, Wout):
    x = np.asarray(x, dtype=np.float32)
    Wq = np.asarray(Wq, dtype=np.float32)
    Wkv = np.asarray(Wkv, dtype=np.float32)
    null_kv = np.asarray(null_kv, dtype=np.float32)
    Wout = np.asarray(Wout, dtype=np.float32)

    scale = DH ** -0.5
    wq_scaled = (Wq * scale).astype(np.float16)
    wkv2 = np.concatenate([Wkv, Wkv], axis=1).astype(np.float16)  # [1024,128]
    nullkv2 = np.concatenate([null_kv, null_kv]).astype(np.float16).reshape(128, 1)
    wout16 = Wout.astype(np.float16)
    ident = np.eye(128, dtype=np.float16)

    # masks[t][i, j] = 1 if j >= i + 128*t - 1  (ST layout: i = k within chunk,
    # j = q within 512 block; delta = 128*t - 1 for diagonal chunk t)
    i_idx = np.arange(128)[:, None]
    j_idx = np.arange(QB)[None, :]
    masks = np.concatenate(
        [(j_idx >= i_idx + 128 * t - 1).astype(np.float16) for t in range(4)], axis=1
    )  # [128, 2048]

    in_maps = []
    for core in range(8):
        b, hg = core // 2, core % 2
        in_maps.append(
            {
                "xt": np.ascontiguousarray(x[b].T).astype(np.float16),
                "wq": wq_scaled[:, hg * 512:(hg + 1) * 512].copy(),
                "wkv2": wkv2,
                "nullkv2": nullkv2,
                "wout": np.ascontiguousarray(wout16[hg * 512:(hg + 1) * 512, :]),
                "masks": masks,
                "ident": ident,
            }
        )
    return in_maps


def kernel(x, Wq, Wkv, null_kv, Wout, _trace=False):
    from concourse import bass_utils

    if "nc" not in _PROGRAM_CACHE:
        _PROGRAM_CACHE["nc"] = _build_program()
    nc = _PROGRAM_CACHE["nc"]

    in_maps = _host_prep(x, Wq, Wkv, null_kv, Wout)
    res = bass_utils.run_bass_kernel_spmd(
        nc, in_maps, core_ids=list(range(8)), trace=_trace
    )
    _PROGRAM_CACHE["last_result"] = res

    outs = [np.asarray(r["out"], dtype=np.float32) for r in res.results]
    full = np.stack([outs[2 * b] + outs[2 * b + 1] for b in range(B)], axis=0)
    return full
